# revision 24
# baseline (speedup 1.0000x reference)
"""Trainium2 Bass kernel for the CriticalField PDE step.

Computes one explicit step of a coupled magnitude/phase field update on a
4096x4096 grid with circular boundary conditions:

    mag_lap   = 4-neighbor circular Laplacian of magnitude
    phase_lap = 4-neighbor circular Laplacian of phase
    d_mag     = tension*mag_lap - damping*mag - nonlinearity*mag^3
    d_phase   = tension*phase_lap + COUPLING*sin(up(phase) - phase)
    out[0]    = clip(mag + DT*d_mag, -2, 2)
    out[1]    = clip(phase + DT*d_phase, 0, 2*pi)

Sharding: rows are split across 8 NeuronCores. Each core processes 504 rows
as 4 tiles of 128 partitions (126 valid output rows each, tiles advance by
126 so the +-1 row stencil reach stays inside the tile), plus 1/8 of the 64
leftover rows (4032..4095) as a column-split "overflow" block. All halos
(row and column, circular) are materialized host-side so the device kernel
needs no collectives and no wrap logic.

Per-core compute strategy (memory-bound target):
  - TensorE: raw 4-neighbor sums + the phase roll-difference via float32r
    matmuls with {0,+-1} banded matrices accumulated in PSUM (row-direction
    neighbors via off-diagonal bands over partitions, column-direction
    neighbors via column-shifted rhs views).
  - ScalarE: square(mag), A2*phase, sin(psum_arg).
  - GpSimd:  -C*mag^3 chain step and the two output clips.
  - VectorE: the three fused scalar_tensor_tensor merges + one.
All scale factors (A, B, ...) are applied as exact fp32 immediates outside
the PE so float32r only ever multiplies data by exactly-representable 1.0.
"""

import numpy as np

SIZE = 4096
NCORES = 8
TILE_VALID = 126
NTILES = 4
MAIN_ROWS = TILE_VALID * NTILES          # 504 rows per core via main tiles
OVF_ROWS = SIZE - MAIN_ROWS * NCORES     # 64 leftover rows (4032..4095)
OVF_COLS = SIZE // NCORES                # 512 columns of overflow per core
DT = 0.05
COUPLING = 0.015
TWO_PI = 2.0 * np.pi

_PROG_CACHE: dict = {}
_WEIGHTS_CACHE: dict = {}


def _banded_weights(tension):
    """lhsT weight matrices for nc.tensor.matmul (out = lhsT.T @ rhs).

    lhsT[k, m] = contribution of rhs partition k to output partition m.
    Output partition m corresponds to slab row t+m; its row-neighbors are
    tile partitions m-1 (up) and m+1 (down). Block 3 is (COUPLING/tension)*I,
    used to inject sin(arg) into the phase PSUM so the whole PSUM can be
    scaled by B = DT*tension in one fused merge.
    """
    key = float(tension)
    if key in _WEIGHTS_CACHE:
        return _WEIGHTS_CACHE[key]
    w_ud = np.zeros((128, 128), np.float32)
    idx = np.arange(127)
    w_ud[idx, idx + 1] = 1.0      # k = m-1 -> up neighbor
    w_ud[idx + 1, idx] = 1.0      # k = m+1 -> down neighbor
    w_eye = np.eye(128, dtype=np.float32)
    w_umi = np.zeros((128, 128), np.float32)
    w_umi[idx, idx + 1] = 1.0     # +up
    w_umi[np.arange(128), np.arange(128)] = -1.0  # -center
    w_sin = np.eye(128, dtype=np.float32) * (COUPLING / tension)
    _WEIGHTS_CACHE[key] = {
        "w_all": np.ascontiguousarray(
            np.concatenate([w_ud, w_eye, w_umi, w_sin], axis=1))}
    return _WEIGHTS_CACHE[key]


def _build_program(A, B, Cc, A2, K, repeat=1, mode="full"):
    import concourse.bass as bass
    import concourse.bacc as bacc
    import concourse.tile as tile
    from concourse import mybir

    f32 = mybir.dt.float32
    f32r = mybir.dt.float32r
    Act = mybir.ActivationFunctionType
    Alu = mybir.AluOpType

    nc = bacc.Bacc(trn_type="TRN2", target_bir_lowering=False, debug=False)

    # Field slabs are declared float32r (same bits as f32) so the PE may
    # consume them directly; non-matmul consumers bitcast back to f32.
    mag_slab = nc.dram_tensor("mag_slab", [MAIN_ROWS + 2, SIZE + 2], f32r,
                              kind="ExternalInput").ap()
    ph_slab = nc.dram_tensor("ph_slab", [MAIN_ROWS + 2, SIZE + 2], f32r,
                             kind="ExternalInput").ap()
    mag_ovf = nc.dram_tensor("mag_ovf", [OVF_ROWS + 2, OVF_COLS + 2], f32r,
                             kind="ExternalInput").ap()
    ph_ovf = nc.dram_tensor("ph_ovf", [OVF_ROWS + 2, OVF_COLS + 2], f32r,
                            kind="ExternalInput").ap()
    w_all_d = nc.dram_tensor("w_all", [128, 512], f32r, kind="ExternalInput").ap()
    out_main = nc.dram_tensor("out_main", [2, MAIN_ROWS, SIZE], f32,
                              kind="ExternalOutput").ap()
    out_ovf = nc.dram_tensor("out_ovf", [2, OVF_ROWS, OVF_COLS], f32,
                             kind="ExternalOutput").ap()

    with tile.TileContext(nc) as tc:
        with (
            tc.tile_pool(name="wts", bufs=1) as wpool,
            tc.tile_pool(name="inp", bufs=2) as inp,
            tc.tile_pool(name="outp", bufs=2) as outp,
            tc.tile_pool(name="tmp", bufs=2) as tmp,
            tc.tile_pool(name="sml", bufs=4) as sml,
            tc.tile_pool(name="psm", bufs=3, space="PSUM") as psm,
            tc.tile_pool(name="psb", bufs=2, space="PSUM") as psb,
        ):
            w_all = wpool.tile([128, 512], f32r, tag="w_all")
            nc.sync.dma_start(w_all[:, :], w_all_d[:, :])

            def emit_block(mg, ph, om, op_, P, ncols):
                """Emit compute for one loaded tile.

                mg/ph: input tiles [P, ncols+2] (col halo included)
                om/op_: output tiles [P, ncols]; valid partitions 1..P-2.
                mode ladder (timing diagnostics): "dma" = loads/stores only;
                "pe" = +matmuls; "peact" = +ScalarE ops; "full" = everything.
                """
                if mode == "dma":
                    nc.vector.tensor_copy(om[0:P, 0:ncols],
                                          mg[0:P, 1:1 + ncols].bitcast(f32))
                    nc.gpsimd.tensor_copy(op_[0:P, 0:ncols],
                                          ph[0:P, 1:1 + ncols].bitcast(f32))
                    return
                do_act = mode in ("peact", "full")
                do_rest = mode == "full"
                wud = w_all[0:P, 0:P]
                weye = w_all[0:P, 128:128 + P]
                wumi = w_all[0:P, 256:256 + P]
                wsin = w_all[0:P, 384:384 + P]
                nblk = (ncols + 1023) // 1024
                for b in range(nblk):
                    b0 = 1024 * b
                    bw = min(1024, ncols - b0)
                    magc = mg[0:P, 1 + b0:1 + b0 + bw].bitcast(f32)
                    phc = ph[0:P, 1 + b0:1 + b0 + bw].bitcast(f32)
                    if do_act:
                        c2 = tmp.tile([P, bw], f32, tag="c2")
                        nc.scalar.activation(c2[:, :], magc, Act.Square,
                                             bias=0.0, scale=float(np.sqrt(Cc)))
                        t2 = tmp.tile([P, bw], f32, tag="t2")
                        nc.scalar.activation(t2[:, :], phc, Act.Copy,
                                             bias=0.0, scale=A2)
                    if do_rest:
                        c3t = tmp.tile([P, bw], f32, tag="c3t")
                        nc.gpsimd.tensor_tensor(
                            c3t[:, :], c2[:, :], magc, Alu.mult)
                        tmg = tmp.tile([P, bw], f32, tag="tmg")
                        nc.vector.scalar_tensor_tensor(
                            tmg[:, :], magc, A, c3t[:, :], Alu.mult, Alu.subtract)

                    for j in range(0, bw, 512):
                        c0 = b0 + j
                        cw = min(512, bw - j)
                        mg_c = mg[0:P, 1 + c0:1 + c0 + cw]
                        mg_l = mg[0:P, c0:c0 + cw]
                        mg_r = mg[0:P, 2 + c0:2 + c0 + cw]
                        ph_c = ph[0:P, 1 + c0:1 + c0 + cw]
                        ph_l = ph[0:P, c0:c0 + cw]
                        ph_r = ph[0:P, 2 + c0:2 + c0 + cw]

                        pm = psm.tile([P, cw], f32, tag="pm")
                        nc.tensor.matmul(pm[:, :], wud, mg_c, start=True, stop=False)
                        nc.tensor.matmul(pm[:, :], weye, mg_l, start=False, stop=False)
                        nc.tensor.matmul(pm[:, :], weye, mg_r, start=False, stop=True)
                        pa = psb.tile([P, cw], f32, tag="pa")
                        nc.tensor.matmul(pa[:, :], wumi, ph_c, start=True, stop=True)
                        pp = psm.tile([P, cw], f32, tag="pp")
                        nc.tensor.matmul(pp[:, :], wud, ph_c, start=True, stop=False)
                        nc.tensor.matmul(pp[:, :], weye, ph_l, start=False, stop=False)
                        if not do_act:
                            nc.tensor.matmul(pp[:, :], weye, ph_r,
                                             start=False, stop=True)
                            continue
                        nc.tensor.matmul(pp[:, :], weye, ph_r,
                                         start=False, stop=False)
                        s = sml.tile([P, cw], f32r, tag="s")
                        nc.scalar.activation(s[:, :], pa[:, :], Act.Sin)
                        nc.tensor.matmul(pp[:, :], wsin, s[:, :],
                                         start=False, stop=True)
                        if not do_rest:
                            continue
                        mm = sml.tile([P, cw], f32, tag="mm")
                        nc.vector.scalar_tensor_tensor(
                            mm[:, :], pm[:, :], B, tmg[:, j:j + cw],
                            Alu.mult, Alu.add)
                        m2a = sml.tile([P, cw], f32, tag="m2a")
                        nc.vector.scalar_tensor_tensor(
                            m2a[:, :], pp[:, :], B, t2[:, j:j + cw],
                            Alu.mult, Alu.add)
                        nc.vector.tensor_scalar(
                            om[0:P, c0:c0 + cw], mm[0:P, :],
                            2.0, -2.0, Alu.min, Alu.max)
                        nc.gpsimd.tensor_scalar(
                            op_[0:P, c0:c0 + cw], m2a[0:P, :],
                            0.0, float(np.float32(TWO_PI)), Alu.max, Alu.min)
                if mode in ("pe", "peact"):
                    nc.vector.tensor_copy(om[0:P, 0:ncols],
                                          mg[0:P, 1:1 + ncols].bitcast(f32))
                    nc.gpsimd.tensor_copy(op_[0:P, 0:ncols],
                                          ph[0:P, 1:1 + ncols].bitcast(f32))

            HALF = SIZE // 2
            for _rep in range(repeat):
              for ti in range(NTILES):
                t0 = TILE_VALID * ti
                # Column-halved tiles: separate tiles (and thus separate DMA
                # completion semaphores) per half, so compute on a fresh tile
                # starts after 1 MB lands instead of 2, and output DMAs drain
                # per-half instead of per-tile.
                for h in range(2):
                    lo = HALF * h
                    mg = inp.tile([128, HALF + 2], f32r, tag=f"mg{h}")
                    nc.sync.dma_start(mg[:, :],
                                      mag_slab[t0:t0 + 128, lo:lo + HALF + 2])
                    ph = inp.tile([128, HALF + 2], f32r, tag=f"ph{h}")
                    nc.sync.dma_start(ph[:, :],
                                      ph_slab[t0:t0 + 128, lo:lo + HALF + 2])
                    om = outp.tile([128, HALF], f32, tag=f"om{h}")
                    op_ = outp.tile([128, HALF], f32, tag=f"op{h}")
                    emit_block(mg, ph, om, op_, 128, HALF)
                    nc.sync.dma_start(
                        out_main[0, t0:t0 + TILE_VALID, lo:lo + HALF],
                        om[1:127, :])
                    nc.sync.dma_start(
                        out_main[1, t0:t0 + TILE_VALID, lo:lo + HALF],
                        op_[1:127, :])

              P = OVF_ROWS + 2
              mg = inp.tile([P, OVF_COLS + 2], f32r, tag="mg0")
              nc.sync.dma_start(mg[:, :], mag_ovf[:, :])
              ph = inp.tile([P, OVF_COLS + 2], f32r, tag="ph0")
              nc.sync.dma_start(ph[:, :], ph_ovf[:, :])
              om = outp.tile([P, OVF_COLS], f32, tag="om0")
              op_ = outp.tile([P, OVF_COLS], f32, tag="op0")
              emit_block(mg, ph, om, op_, P, OVF_COLS)
              nc.sync.dma_start(out_ovf[0, :, :], om[1:P - 1, :])
              nc.sync.dma_start(out_ovf[1, :, :], op_[1:P - 1, :])

    nc.compile()
    return nc


def _get_program(damping, tension, nonlinearity, repeat=1, mode="full"):
    key = (damping, tension, nonlinearity, repeat, mode)
    if key not in _PROG_CACHE:
        A = 1.0 - 4.0 * DT * tension - DT * damping
        B = DT * tension
        Cc = DT * nonlinearity
        A2 = 1.0 - 4.0 * DT * tension
        K = DT * COUPLING
        _PROG_CACHE[key] = _build_program(A, B, Cc, A2, K, repeat, mode)
    return _PROG_CACHE[key]


def _make_in_maps(mag, ph, tension=1.5):
    """Build per-core input dicts with all circular halos materialized."""
    w = _banded_weights(tension)
    cols = np.arange(-1, SIZE + 1) % SIZE
    ovf_rows = np.arange(MAIN_ROWS * NCORES - 1, SIZE + 1) % SIZE
    mag_ovf_full = mag[np.ix_(ovf_rows, cols)]
    ph_ovf_full = ph[np.ix_(ovf_rows, cols)]
    in_maps = []
    for m in range(NCORES):
        rows = np.arange(MAIN_ROWS * m - 1, MAIN_ROWS * (m + 1) + 1) % SIZE
        c0 = OVF_COLS * m
        in_maps.append({
            "mag_slab": np.ascontiguousarray(mag[np.ix_(rows, cols)]),
            "ph_slab": np.ascontiguousarray(ph[np.ix_(rows, cols)]),
            "mag_ovf": np.ascontiguousarray(mag_ovf_full[:, c0:c0 + OVF_COLS + 2]),
            "ph_ovf": np.ascontiguousarray(ph_ovf_full[:, c0:c0 + OVF_COLS + 2]),
            "w_all": w["w_all"],
        })
    return in_maps


def _assemble(results):
    out = np.empty((1, 2, SIZE, SIZE), np.float32)
    for m in range(NCORES):
        r = results[m]
        out[0, :, MAIN_ROWS * m:MAIN_ROWS * (m + 1), :] = r["out_main"]
        out[0, :, MAIN_ROWS * NCORES:, OVF_COLS * m:OVF_COLS * (m + 1)] = \
            r["out_ovf"]
    return out


def kernel(magnitude, phase, damping, tension, nonlinearity):
    from concourse.bass_utils import run_bass_kernel_spmd

    mag = np.asarray(magnitude, dtype=np.float32).reshape(SIZE, SIZE)
    ph = np.asarray(phase, dtype=np.float32).reshape(SIZE, SIZE)
    d = float(np.asarray(damping))
    tn = float(np.asarray(tension))
    nl = float(np.asarray(nonlinearity))

    nc = _get_program(d, tn, nl)
    in_maps = _make_in_maps(mag, ph, tn)
    res = run_bass_kernel_spmd(nc, in_maps, core_ids=list(range(NCORES)))
    return _assemble(res.results)
